# revision 15
# baseline (speedup 1.0000x reference)
"""Multi-head self-attention Trainium2 kernel (8 NeuronCores, head-parallel).

Problem: L=4096, F_IN=1024, H=16, DH=64, F_OUT=1024, fp32.
Sharding: 2 heads per core (tensor parallel over heads). Each core computes
its 2 heads' attention and its partial output projection; the host sums the
8 partials (the all-reduce of the sharding hint, done at gather time).

v2 changes vs the 394us baseline:
  * fp16 everywhere in SBUF (x, q, k, v, Wo, out): fp16 matmuls stream at
    1 cycle/row (the fp32r oproj/proj moving operands were 2 cyc/row), DMA
    bytes halve, and fp16's 2^-11 rounding buys error budget for the
    Schraudolph tiles below.
  * ~25% of the 256 exp tiles move off ScalarE (the old roofline at
    1.11us/tile) onto the idle VectorE as a one-instruction exp bit-trick:
    i16 = rint(A*s + B) written into the fp16 eT tile via an int16 bitcast
    view, where A = 0.125*log2(e)*1024, B = 15360-59. The int16 value IS
    the fp16 bit pattern of ~exp(s/8) (max rel err 4%, RMS 1.8%). The
    softmax denominator sums the same approximated weights (ones-column
    trick), so normalization stays consistent; only the weight *shape*
    within DVE-assigned j-tiles carries the sawtooth noise.
  * Output projection packs both heads into one K=128 matmul (normalized
    vals for head0/head1 land in rows 0:64/64:128 of one fp16 tile; Wo is
    host-packed to [128, F_OUT]), halving oproj matmuls and enabling FWL.
  * Phase 1 runs in 8 chunks of 512 with one batched x DMA per chunk
    (the old per-[128,512]-fp32 DMAs serialized ~40us on the sync queue).

v3 changes vs v2 (386us):
  * The DVE queue is 8-deep strict FIFO, and v2 clogged it: sem-waits and
    oproj evacuations sat ahead of the critical Schraudolph exps, which
    stalled vals -> scores-buffer reuse -> the ScalarE ACT stream (~123us
    of ACT gaps). v3 dedicates the DVE queue to exp + psum evacuations:
    - the Schraudolph exp for step n+1 is emitted one step early (right
      after its scores matmuls), so it runs concurrently with ACT(n);
    - the normalize chain's DMAs (denominator extract, reciprocal
      broadcast) move to the GpSimd DGE queue, off the Sync queue that
      out-DMAs block, and the gpsimd partition_broadcast becomes a
      stride-0-partition DMA so GpSimd<->DVE sem ping-pong disappears.
  * exp offload raised to 10/32 j-tiles (~31%).
  * Startup: x chunk-0 DMA issues before the wq/wo DMAs (first proj matmul
    ~12us earlier); tail: the final flush alternates its psum evacuations
    between ScalarE and VectorE so the 2-buffer pso chain pipelines.

Per-core pipeline:
  1. qT,kT [128,4096] fp16 = W.T @ x.T; v via PE transpose-mode into
     natural [j, d] layout vx (fp16, ones column appended for the softmax
     denominator).
  2. Per i-chunk (512), per j-tile (128): scoresT for both heads as
     row-packed K=64 matmuls -> exp over the [128,1024] psum pair on
     ScalarE (ACT, fp16 out) or VectorE (Schraudolph) -> attn@v with the
     ones-column so PSUM row 64 accumulates the denominators.
  3. Normalize + output projection interleave into the next chunk's j-loop:
     evacuate vals fp32, reciprocal_approx_fast on the DMA-extracted
     denominators, gpsimd broadcast, normalize into packed fp16 rows,
     one K=128 matmul per (i-tile, f-chunk), fp16 DMA out.

Bias handling: bq/bk folded into the ACT bias at qT/kT evacuation; bv exact
on host (softmax rows sum to 1 => out += sum_h bv_h @ Wo_h); bo on host.
"""

import numpy as np

L, F_IN, H, DH, F_OUT = 4096, 1024, 16, 64, 1024
NCORES = 8
HPC = H // NCORES  # heads per core = 2
D2 = HPC * DH      # 128, per-core packed head dim

_BUILT = None

# fp16-bit-space Schraudolph exp: bits = rint(A*s + B) ~ fp16(exp(0.125*s))
A_SCH = 0.125 * float(np.log2(np.e)) * 1024.0
B_SCH = 15360.0 - 59.0
# j-tiles whose exp runs on VectorE (rest on ScalarE)
DVE_JT = frozenset(jt for jt in range(32) if jt % 3 == 2)


def _build():
    import concourse.bass as bass  # noqa: F401
    import concourse.mybir as mybir
    import concourse.tile as tile
    from concourse import bacc
    from concourse.masks import make_identity

    F = mybir.dt.float32
    HF = mybir.dt.float16
    I16 = mybir.dt.int16
    Act = mybir.ActivationFunctionType

    nc = bacc.Bacc("TRN2", target_bir_lowering=False, debug=False)

    xT_d = nc.declare_dram_parameter("xT", [F_IN, L], HF, isOutput=False)
    wq_d = nc.declare_dram_parameter("wq", [F_IN, D2], HF, isOutput=False)
    wk_d = nc.declare_dram_parameter("wk", [F_IN, D2], HF, isOutput=False)
    wv_d = nc.declare_dram_parameter("wv", [F_IN, D2], HF, isOutput=False)
    bq_d = nc.declare_dram_parameter("bq", [D2], F, isOutput=False)
    bk_d = nc.declare_dram_parameter("bk", [D2], F, isOutput=False)
    wo_d = nc.declare_dram_parameter("wo", [D2, F_OUT], HF, isOutput=False)
    out_d = nc.declare_dram_parameter("out", [L, F_OUT], HF, isOutput=True)

    KT = F_IN // 128   # 8 f-tiles
    NI = L // 512      # 8 i-chunks
    NJ = L // 128      # 32 j-tiles
    CL = 512           # phase-1 chunk width in L
    NC = L // CL       # 8 chunks

    with tile.TileContext(nc) as tc:
        with tc.tile_pool(name="persist", bufs=1) as pp:
            qT = pp.tile([128, L], HF, tag="qT")             # [d2, i]
            kT = pp.tile([128, L], HF, tag="kT")             # [d2, j]
            vx0 = pp.tile([128, NJ, DH + 1], HF, tag="vx0")  # [j_in, jt, d|1]
            vx1 = pp.tile([128, NJ, DH + 1], HF, tag="vx1")
            bq = pp.tile([128, 1], F, tag="bq")
            bk = pp.tile([128, 1], F, tag="bk")
            ones32 = pp.tile([128, NJ], HF, tag="ones32")
            warm = pp.tile([1, 1], F, tag="warm")
            warm2 = pp.tile([1, 1], F, tag="warm2")

            # pre-warm the exp table set + the gpsimd DGE while DMAs run
            nc.vector.memset(warm[:], 0.0)
            nc.scalar.activation(warm[:], warm[:], Act.Exp, scale=1.0)
            nc.gpsimd.dma_start(out=warm2[:], in_=warm[:])

            nc.vector.memset(ones32[:], 1.0)
            nc.vector.tensor_copy(vx0[:, :, DH:DH + 1], ones32[:, :, None])
            nc.vector.tensor_copy(vx1[:, :, DH:DH + 1], ones32[:, :, None])

            # Pools for the attention phase are opened before phase 1 is
            # emitted so the scheduler can overlap the phase-1 tail with
            # early score matmuls (PSUM: ps2s 4 + ps2v 2 + phase1 2 = 8).
            with tc.tile_pool(name="p2", bufs=1) as p2, \
                 tc.tile_pool(name="p2v", bufs=2) as p2v, \
                 tc.tile_pool(name="expp", bufs=6) as pe, \
                 tc.tile_pool(name="outp", bufs=4) as po, \
                 tc.tile_pool(name="ps2s", bufs=2, space="PSUM") as ps2s, \
                 tc.tile_pool(name="ps2v", bufs=1, space="PSUM") as ps2v:
                wo = p2.tile([D2, F_OUT], HF, tag="wo")

                # ---- Phase 1: QKV projections over 8 chunks of L ----
                with tc.tile_pool(name="p1w", bufs=1) as p1w, \
                     tc.tile_pool(name="p1x", bufs=2) as p1x, \
                     tc.tile_pool(name="ps1", bufs=2, space="PSUM") as ps1:
                    wq = p1w.tile([128, KT, D2], HF, tag="wq")
                    wk = p1w.tile([128, KT, D2], HF, tag="wk")
                    wv = p1w.tile([128, KT, D2], HF, tag="wv")
                    ident = p1w.tile([128, 128], HF, tag="ident")
                    # DMA order matters for the pipeline ramp: the first
                    # k-projection needs wk + x chunk 0, so everything else
                    # queues behind those two.
                    nc.sync.dma_start(
                        out=wk[:],
                        in_=wk_d.ap().rearrange("(k p) d -> p k d", p=128),
                    )
                    make_identity(nc, ident[:])
                    nc.sync.dma_start(out=bk[:], in_=bk_d.ap()[:, None])
                    nc.sync.dma_start(out=bq[:], in_=bq_d.ap()[:, None])

                    def proj(wt, dst, bias, xt, g0):
                        ps = ps1.tile([128, 512], F, tag="ps1")
                        for kt in range(KT):
                            nc.tensor.matmul(
                                ps[:], wt[:, kt, :], xt[:, kt, :],
                                start=(kt == 0), stop=(kt == KT - 1),
                            )
                        if bias is not None:
                            nc.scalar.activation(
                                dst[:, g0:g0 + 512], ps[:], Act.Identity,
                                bias=bias[:], scale=1.0,
                            )
                        else:
                            nc.scalar.copy(dst[:, 0:512], ps[:])

                    for ch in range(NC):
                        l0 = ch * CL
                        xt = p1x.tile([128, KT, CL], HF, tag="xt")
                        nc.sync.dma_start(
                            out=xt[:],
                            in_=xT_d.ap().rearrange(
                                "(k p) l -> p k l", p=128)[:, :, l0:l0 + CL],
                        )
                        if ch == 0:
                            for wt, wd in ((wv, wv_d), (wq, wq_d)):
                                nc.sync.dma_start(
                                    out=wt[:],
                                    in_=wd.ap().rearrange(
                                        "(k p) d -> p k d", p=128),
                                )
                            nc.sync.dma_start(out=wo[:], in_=wo_d.ap())
                        vTq = p1x.tile([128, CL], HF, tag="vTq")
                        # k and v unlock this chunk's score matmuls; qT is
                        # ramp-critical only in chunk 0 (i-chunk 0 columns),
                        # so later chunks emit the q projection last.
                        proj(wk, kT, bk, xt, l0)
                        proj(wv, vTq, None, xt, l0)
                        if ch == 0:
                            proj(wq, qT, bq, xt, l0)
                        for jl in range(CL // 128):
                            jt = ch * (CL // 128) + jl
                            pt = ps1.tile([128, 1024], HF, tag="ps1")
                            nc.tensor.transpose(
                                pt[:, 0:128],
                                vTq[:, jl * 128:(jl + 1) * 128], ident[:])
                            nc.vector.tensor_copy(vx0[:, jt, 0:DH], pt[:, 0:DH])
                            nc.vector.tensor_copy(vx1[:, jt, 0:DH],
                                                  pt[:, DH:D2])
                        if ch != 0:
                            proj(wq, qT, bq, xt, l0)

                # ---- Phase 2+3: attention, interleaved normalize/out-proj ----
                with tc.tile_pool(name="ps2o", bufs=2, space="PSUM") as ps2o:
                    _phase2(nc, ps2s, ps2v, ps2o, pe, po, p2v,
                            qT, kT, vx0, vx1, wo, out_d,
                            NI, NJ, F, HF, I16, Act, mybir)

    nc.compile()
    return nc


def _phase2(nc, ps2s, ps2v, ps2o, pe, po, p2v, qT, kT, vx0, vx1, wo,
            out_d, NI, NJ, F, HF, I16, Act, mybir):
    def norm_a(ic, va0, va1):
        # denominator extract + reciprocal for chunk ic. The extraction
        # DMAs ride the gpsimd DGE queue so out-DMA sem waits on the sync
        # queue cannot delay them.
        sh = p2v.tile([1, 1024], F, tag="sh")
        rc = p2v.tile([1, 1024], F, tag="rc")
        nc.gpsimd.dma_start(out=sh[:, 0:512], in_=va0[DH:DH + 1, :])
        nc.gpsimd.dma_start(out=sh[:, 512:1024], in_=va1[DH:DH + 1, :])
        nc.vector.reciprocal_approx_fast(out=rc[:], in_=sh[:])
        return rc

    def norm_b(ic, rc, va0, va1, van):
        # broadcast + normalize into packed fp16 rows (h0 0:64, h1 64:128),
        # emitted one step after norm_a so the gpsimd/DVE handoffs never
        # head-block the DVE queue ahead of a prefetched exp.
        for va, r0, c0, tg in ((va0, 0, 0, "0"), (va1, DH, 512, "1")):
            rb = p2v.tile([DH, 512], F, tag="rb" + tg)
            nc.gpsimd.partition_broadcast(rb[:], rc[0:1, c0:c0 + 512],
                                          channels=DH)
            nc.vector.tensor_mul(van[r0:r0 + DH, :], va[0:DH, :], rb[:])

    def oproj_unit(ic, van, iw, fc, evac="v"):
        # one output-projection tile of chunk ic: both heads in one K=128 mm
        isl = slice(iw * 128, (iw + 1) * 128)
        r0 = ic * 512 + iw * 128
        f0 = fc * 512
        pso = ps2o.tile([128, 512], F, tag="pso")
        nc.tensor.matmul(
            pso[:], van[:, isl], wo[:, f0:f0 + 512],
            start=True, stop=True,
        )
        ot = po.tile([128, 512], HF, tag="ot")
        if evac == "s":
            nc.scalar.copy(ot[:], pso[:])
        else:
            nc.vector.tensor_copy(ot[:], pso[:])
        nc.sync.dma_start(
            out=out_d.ap()[r0:r0 + 128, f0:f0 + 512], in_=ot[:])

    def emit_scores(ic, jt):
        i0 = ic * 512
        j0 = jt * 128
        ps = ps2s.tile([128, 1024], F, tag="pss")
        nc.tensor.matmul(
            ps[:, 0:512], kT[0:64, j0:j0 + 128], qT[0:64, i0:i0 + 512],
            start=True, stop=True, tile_position=(0, 0),
        )
        nc.tensor.matmul(
            ps[:, 512:1024], kT[64:128, j0:j0 + 128],
            qT[64:128, i0:i0 + 512],
            start=True, stop=True, tile_position=(64, 0),
        )
        return ps

    def emit_exp(ps, eT, jt):
        if jt in DVE_JT:
            nc.vector.tensor_scalar(
                out=eT[:].bitcast(I16),
                in0=ps[:],
                scalar1=A_SCH,
                scalar2=B_SCH,
                op0=mybir.AluOpType.mult,
                op1=mybir.AluOpType.add,
            )
        else:
            nc.scalar.activation(eT[:], ps[:], Act.Exp, scale=0.125)

    # One-step score skew: scores for step n+1 are emitted between exp(n)
    # and vals(n), so on TensorE's FIFO they are not blocked behind the
    # exp(n)-gated vals, and exp(n+1)'s input is ready a full step early.
    # DVE-assigned exps are additionally emitted one step early (right
    # after their scores), so they run concurrently with ACT(n) instead of
    # queueing behind step-n DVE work.
    pending = None  # (ic, va0, va1, van) with normalize+out-proj still to emit
    ps_next = emit_scores(0, 0)
    eT_next = None
    for ic in range(NI):
        pv0 = ps2v.tile([DH + 1, 512], F, tag="pv0")
        pv1 = ps2v.tile([DH + 1, 512], F, tag="pv1")
        # out-proj units of the previous chunk, spread through this jt loop
        units = []
        if pending is not None:
            pic, _, _, pvan = pending
            units = [(pic, pvan, iw, fc)
                     for iw in range(4) for fc in range(F_OUT // 512)]
        for jt in range(NJ):
            ps = ps_next
            if eT_next is not None:
                eT = eT_next
                eT_next = None
            else:
                eT = pe.tile([128, 1024], HF, tag="eT")
                emit_exp(ps, eT, jt)
            n = ic * NJ + jt
            if n + 1 < NI * NJ:
                jt_next = (n + 1) % NJ
                ps_next = emit_scores((n + 1) // NJ, jt_next)
                if jt_next in DVE_JT:
                    eT_next = pe.tile([128, 1024], HF, tag="eT")
                    emit_exp(ps_next, eT_next, jt_next)
            nc.tensor.matmul(
                pv0[:], vx0[:, jt, :], eT[:, 0:512],
                start=(jt == 0), stop=(jt == NJ - 1),
            )
            nc.tensor.matmul(
                pv1[:], vx1[:, jt, :], eT[:, 512:1024],
                start=(jt == 0), stop=(jt == NJ - 1),
            )
            if pending is not None:
                if jt == 0:
                    rc_pend = norm_a(pending[0], pending[1], pending[2])
                elif jt == 1:
                    norm_b(pending[0], rc_pend, pending[1], pending[2],
                           pending[3])
            if jt % 2 == 1 and 3 <= jt < 19 and units:
                oproj_unit(*units.pop(0))
        for u in units:
            oproj_unit(*u)

        # evacuate this chunk's vals+denominators: the next chunk's first
        # vals matmul reclaims the PSUM bank after one copy per head
        va0 = p2v.tile([DH + 1, 512], F, tag="va0")
        va1 = p2v.tile([DH + 1, 512], F, tag="va1")
        van = p2v.tile([D2, 512], HF, tag="van")
        nc.vector.tensor_copy(va0[:], pv0[:])
        nc.vector.tensor_copy(va1[:], pv1[:])
        pending = (ic, va0, va1, van)

    # final flush: alternate psum evacuation engines so the 2-buffer pso
    # chain pipelines instead of serializing on the DVE
    rc_fin = norm_a(pending[0], pending[1], pending[2])
    norm_b(pending[0], rc_fin, pending[1], pending[2], pending[3])
    k = 0
    for iw in range(4):
        for fc in range(F_OUT // 512):
            oproj_unit(pending[0], pending[3], iw, fc,
                       evac="s" if k % 2 else "v")
            k += 1


def _get_built():
    global _BUILT
    if _BUILT is None:
        _BUILT = _build()
    return _BUILT


def kernel(x, Wq, bq, Wk, bk, Wv, bv, Wo, bo):
    from concourse.bass_utils import run_bass_kernel_spmd

    x = np.asarray(x, dtype=np.float32)
    Wq = np.asarray(Wq, dtype=np.float32)
    Wk = np.asarray(Wk, dtype=np.float32)
    Wv = np.asarray(Wv, dtype=np.float32)
    Wo = np.asarray(Wo, dtype=np.float32)
    bq = np.asarray(bq, dtype=np.float32)
    bk = np.asarray(bk, dtype=np.float32)
    bv = np.asarray(bv, dtype=np.float32)
    bo = np.asarray(bo, dtype=np.float32)

    nc = _get_built()

    xT = np.ascontiguousarray(x.T.astype(np.float16))  # [F_IN, L]
    in_maps = []
    for c in range(NCORES):
        hs = slice(c * HPC, (c + 1) * HPC)
        in_maps.append({
            "xT": xT,
            "wq": np.ascontiguousarray(
                Wq[:, hs, :].reshape(F_IN, D2).astype(np.float16)),
            "wk": np.ascontiguousarray(
                Wk[:, hs, :].reshape(F_IN, D2).astype(np.float16)),
            "wv": np.ascontiguousarray(
                Wv[:, hs, :].reshape(F_IN, D2).astype(np.float16)),
            "bq": np.ascontiguousarray(bq[hs].reshape(D2)),
            "bk": np.ascontiguousarray(bk[hs].reshape(D2)),
            "wo": np.ascontiguousarray(
                Wo[hs].reshape(D2, F_OUT).astype(np.float16)),
        })

    res = run_bass_kernel_spmd(nc, in_maps, list(range(NCORES)))
    acc = np.zeros((L, F_OUT), dtype=np.float64)
    for c in range(NCORES):
        acc += res.results[c]["out"].astype(np.float64)
    # bv contribution (softmax rows sum to 1) + bo, both exact on host
    acc += (bv.reshape(1, H * DH).astype(np.float64)
            @ Wo.reshape(H * DH, F_OUT).astype(np.float64))
    acc += bo.astype(np.float64)
    return acc.astype(np.float32)


# revision 17
# speedup vs baseline: 1.0755x; 1.0755x over previous
"""Multi-head self-attention Trainium2 kernel (8 NeuronCores, head-parallel).

Problem: L=4096, F_IN=1024, H=16, DH=64, F_OUT=1024, fp32.
Sharding: 2 heads per core (tensor parallel over heads). Each core computes
its 2 heads' attention and its partial output projection; the host sums the
8 partials (the all-reduce of the sharding hint, done at gather time).

v2 changes vs the 394us baseline:
  * fp16 everywhere in SBUF (x, q, k, v, Wo, out): fp16 matmuls stream at
    1 cycle/row (the fp32r oproj/proj moving operands were 2 cyc/row), DMA
    bytes halve, and fp16's 2^-11 rounding buys error budget for the
    Schraudolph tiles below.
  * ~25% of the 256 exp tiles move off ScalarE (the old roofline at
    1.11us/tile) onto the idle VectorE as a one-instruction exp bit-trick:
    i16 = rint(A*s + B) written into the fp16 eT tile via an int16 bitcast
    view, where A = 0.125*log2(e)*1024, B = 15360-59. The int16 value IS
    the fp16 bit pattern of ~exp(s/8) (max rel err 4%, RMS 1.8%). The
    softmax denominator sums the same approximated weights (ones-column
    trick), so normalization stays consistent; only the weight *shape*
    within DVE-assigned j-tiles carries the sawtooth noise.
  * Output projection packs both heads into one K=128 matmul (normalized
    vals for head0/head1 land in rows 0:64/64:128 of one fp16 tile; Wo is
    host-packed to [128, F_OUT]), halving oproj matmuls and enabling FWL.
  * Phase 1 runs in 8 chunks of 512 with one batched x DMA per chunk
    (the old per-[128,512]-fp32 DMAs serialized ~40us on the sync queue).

v3 changes vs v2 (386us):
  * The DVE queue is 8-deep strict FIFO, and v2 clogged it: sem-waits and
    oproj evacuations sat ahead of the critical Schraudolph exps, which
    stalled vals -> scores-buffer reuse -> the ScalarE ACT stream (~123us
    of ACT gaps). v3 dedicates the DVE queue to exp + psum evacuations:
    - the Schraudolph exp for step n+1 is emitted one step early (right
      after its scores matmuls), so it runs concurrently with ACT(n);
    - the normalize chain's DMAs (denominator extract, reciprocal
      broadcast) move to the GpSimd DGE queue, off the Sync queue that
      out-DMAs block, and the gpsimd partition_broadcast becomes a
      stride-0-partition DMA so GpSimd<->DVE sem ping-pong disappears.
  * exp offload raised to 10/32 j-tiles (~31%).
  * Startup: x chunk-0 DMA issues before the wq/wo DMAs (first proj matmul
    ~12us earlier); tail: the final flush alternates its psum evacuations
    between ScalarE and VectorE so the 2-buffer pso chain pipelines.

Per-core pipeline:
  1. qT,kT [128,4096] fp16 = W.T @ x.T; v via PE transpose-mode into
     natural [j, d] layout vx (fp16, ones column appended for the softmax
     denominator).
  2. Per i-chunk (512), per j-tile (128): scoresT for both heads as
     row-packed K=64 matmuls -> exp over the [128,1024] psum pair on
     ScalarE (ACT, fp16 out) or VectorE (Schraudolph) -> attn@v with the
     ones-column so PSUM row 64 accumulates the denominators.
  3. Normalize + output projection interleave into the next chunk's j-loop:
     evacuate vals fp32, reciprocal_approx_fast on the DMA-extracted
     denominators, gpsimd broadcast, normalize into packed fp16 rows,
     one K=128 matmul per (i-tile, f-chunk), fp16 DMA out.

Bias handling: bq/bk folded into the ACT bias at qT/kT evacuation; bv exact
on host (softmax rows sum to 1 => out += sum_h bv_h @ Wo_h); bo on host.
"""

import numpy as np

L, F_IN, H, DH, F_OUT = 4096, 1024, 16, 64, 1024
NCORES = 8
HPC = H // NCORES  # heads per core = 2
D2 = HPC * DH      # 128, per-core packed head dim

_BUILT = None

# fp16-bit-space Schraudolph exp: bits = rint(A*s + B) ~ fp16(exp(0.125*s))
A_SCH = 0.125 * float(np.log2(np.e)) * 1024.0
B_SCH = 15360.0 - 59.0
# j-tiles whose exp runs on VectorE (rest on ScalarE)
DVE_JT = frozenset(jt for jt in range(32) if jt % 3 == 2)


def _build():
    import concourse.bass as bass  # noqa: F401
    import concourse.mybir as mybir
    import concourse.tile as tile
    from concourse import bacc
    from concourse.masks import make_identity

    F = mybir.dt.float32
    HF = mybir.dt.float16
    I16 = mybir.dt.int16
    Act = mybir.ActivationFunctionType

    nc = bacc.Bacc("TRN2", target_bir_lowering=False, debug=False)

    xT_d = nc.declare_dram_parameter("xT", [F_IN, L], HF, isOutput=False)
    wq_d = nc.declare_dram_parameter("wq", [F_IN, D2], HF, isOutput=False)
    wk_d = nc.declare_dram_parameter("wk", [F_IN, D2], HF, isOutput=False)
    wv_d = nc.declare_dram_parameter("wv", [F_IN, D2], HF, isOutput=False)
    bq_d = nc.declare_dram_parameter("bq", [D2], F, isOutput=False)
    bk_d = nc.declare_dram_parameter("bk", [D2], F, isOutput=False)
    wo_d = nc.declare_dram_parameter("wo", [D2, F_OUT], HF, isOutput=False)
    out_d = nc.declare_dram_parameter("out", [L, F_OUT], HF, isOutput=True)

    KT = F_IN // 128   # 8 f-tiles
    NI = L // 512      # 8 i-chunks
    NJ = L // 128      # 32 j-tiles
    CL = 512           # phase-1 chunk width in L
    NC = L // CL       # 8 chunks

    with tile.TileContext(nc) as tc:
        with tc.tile_pool(name="persist", bufs=1) as pp:
            qT = pp.tile([128, L], HF, tag="qT")             # [d2, i]
            kT = pp.tile([128, L], HF, tag="kT")             # [d2, j]
            vx0 = pp.tile([128, NJ, DH + 1], HF, tag="vx0")  # [j_in, jt, d|1]
            vx1 = pp.tile([128, NJ, DH + 1], HF, tag="vx1")
            bq = pp.tile([128, 1], F, tag="bq")
            bk = pp.tile([128, 1], F, tag="bk")
            ones32 = pp.tile([128, NJ], HF, tag="ones32")
            warm = pp.tile([1, 1], F, tag="warm")
            warm2 = pp.tile([1, 1], F, tag="warm2")

            # pre-warm the exp table set + the gpsimd DGE while DMAs run
            nc.vector.memset(warm[:], 0.0)
            nc.scalar.activation(warm[:], warm[:], Act.Exp, scale=1.0)
            nc.gpsimd.dma_start(out=warm2[:], in_=warm[:])

            nc.vector.memset(ones32[:], 1.0)
            nc.vector.tensor_copy(vx0[:, :, DH:DH + 1], ones32[:, :, None])
            nc.vector.tensor_copy(vx1[:, :, DH:DH + 1], ones32[:, :, None])

            # Pools for the attention phase are opened before phase 1 is
            # emitted so the scheduler can overlap the phase-1 tail with
            # early score matmuls (PSUM: ps2s 4 + ps2v 2 + phase1 2 = 8).
            with tc.tile_pool(name="p2", bufs=1) as p2, \
                 tc.tile_pool(name="p2v", bufs=2) as p2v, \
                 tc.tile_pool(name="expp", bufs=6) as pe, \
                 tc.tile_pool(name="outp", bufs=4) as po, \
                 tc.tile_pool(name="ps2s", bufs=2, space="PSUM") as ps2s, \
                 tc.tile_pool(name="ps2v", bufs=1, space="PSUM") as ps2v:
                wo = p2.tile([D2, F_OUT], HF, tag="wo")

                # ---- Phase 1: QKV projections over 8 chunks of L ----
                with tc.tile_pool(name="p1w", bufs=1) as p1w, \
                     tc.tile_pool(name="p1x", bufs=2) as p1x, \
                     tc.tile_pool(name="ps1", bufs=2, space="PSUM") as ps1:
                    wq = p1w.tile([128, KT, D2], HF, tag="wq")
                    wk = p1w.tile([128, KT, D2], HF, tag="wk")
                    wv = p1w.tile([128, KT, D2], HF, tag="wv")
                    ident = p1w.tile([128, 128], HF, tag="ident")
                    # DMA order matters for the pipeline ramp: the first
                    # k-projection needs wk + x chunk 0, so everything else
                    # queues behind those two.
                    nc.sync.dma_start(
                        out=wk[:],
                        in_=wk_d.ap().rearrange("(k p) d -> p k d", p=128),
                    )
                    make_identity(nc, ident[:])
                    nc.sync.dma_start(out=bk[:], in_=bk_d.ap()[:, None])
                    nc.sync.dma_start(out=bq[:], in_=bq_d.ap()[:, None])

                    def proj(wt, dst, bias, xt, g0):
                        ps = ps1.tile([128, 512], F, tag="ps1")
                        for kt in range(KT):
                            nc.tensor.matmul(
                                ps[:], wt[:, kt, :], xt[:, kt, :],
                                start=(kt == 0), stop=(kt == KT - 1),
                            )
                        if bias is not None:
                            nc.scalar.activation(
                                dst[:, g0:g0 + 512], ps[:], Act.Identity,
                                bias=bias[:], scale=1.0,
                            )
                        else:
                            nc.scalar.copy(dst[:, 0:512], ps[:])

                    for ch in range(NC):
                        l0 = ch * CL
                        xt = p1x.tile([128, KT, CL], HF, tag="xt")
                        nc.sync.dma_start(
                            out=xt[:],
                            in_=xT_d.ap().rearrange(
                                "(k p) l -> p k l", p=128)[:, :, l0:l0 + CL],
                        )
                        if ch == 0:
                            for wt, wd in ((wv, wv_d), (wq, wq_d)):
                                nc.sync.dma_start(
                                    out=wt[:],
                                    in_=wd.ap().rearrange(
                                        "(k p) d -> p k d", p=128),
                                )
                            nc.sync.dma_start(out=wo[:], in_=wo_d.ap())
                        vTq = p1x.tile([128, CL], HF, tag="vTq")
                        # k and v unlock this chunk's score matmuls; qT is
                        # ramp-critical only in chunk 0 (i-chunk 0 columns),
                        # so later chunks emit the q projection last.
                        proj(wk, kT, bk, xt, l0)
                        proj(wv, vTq, None, xt, l0)
                        if ch == 0:
                            proj(wq, qT, bq, xt, l0)
                        for jl in range(CL // 128):
                            jt = ch * (CL // 128) + jl
                            pt = ps1.tile([128, 1024], HF, tag="ps1")
                            nc.tensor.transpose(
                                pt[:, 0:128],
                                vTq[:, jl * 128:(jl + 1) * 128], ident[:])
                            nc.vector.tensor_copy(vx0[:, jt, 0:DH], pt[:, 0:DH])
                            nc.vector.tensor_copy(vx1[:, jt, 0:DH],
                                                  pt[:, DH:D2])
                        if ch != 0:
                            proj(wq, qT, bq, xt, l0)

                # ---- Phase 2+3: attention, interleaved normalize/out-proj ----
                with tc.tile_pool(name="ps2o", bufs=2, space="PSUM") as ps2o:
                    _phase2(nc, ps2s, ps2v, ps2o, pe, po, p2v,
                            qT, kT, vx0, vx1, wo, out_d,
                            NI, NJ, F, HF, I16, Act, mybir)

    nc.compile()
    return nc


def _phase2(nc, ps2s, ps2v, ps2o, pe, po, p2v, qT, kT, vx0, vx1, wo,
            out_d, NI, NJ, F, HF, I16, Act, mybir):
    def norm_a(ic, va0, va1):
        # denominator extract + reciprocal for chunk ic. The extraction
        # DMAs ride the gpsimd DGE queue so out-DMA sem waits on the sync
        # queue cannot delay them.
        sh = p2v.tile([1, 1024], F, tag="sh")
        rc = p2v.tile([1, 1024], F, tag="rc")
        nc.gpsimd.dma_start(out=sh[:, 0:512], in_=va0[DH:DH + 1, :])
        nc.gpsimd.dma_start(out=sh[:, 512:1024], in_=va1[DH:DH + 1, :])
        nc.vector.reciprocal_approx_fast(out=rc[:], in_=sh[:])
        return rc

    def norm_b(ic, rc, va0, va1, van):
        # broadcast + normalize into packed fp16 rows (h0 0:64, h1 64:128),
        # emitted one step after norm_a so the gpsimd/DVE handoffs never
        # head-block the DVE queue ahead of a prefetched exp.
        for va, r0, c0, tg in ((va0, 0, 0, "0"), (va1, DH, 512, "1")):
            rb = p2v.tile([DH, 512], F, tag="rb" + tg)
            nc.gpsimd.partition_broadcast(rb[:], rc[0:1, c0:c0 + 512],
                                          channels=DH)
            nc.vector.tensor_mul(van[r0:r0 + DH, :], va[0:DH, :], rb[:])

    def oproj_unit(ic, van, iw, fc, evac="v"):
        # one output-projection tile of chunk ic: both heads in one K=128 mm
        isl = slice(iw * 128, (iw + 1) * 128)
        r0 = ic * 512 + iw * 128
        f0 = fc * 512
        pso = ps2o.tile([128, 512], F, tag="pso")
        nc.tensor.matmul(
            pso[:], van[:, isl], wo[:, f0:f0 + 512],
            start=True, stop=True,
        )
        ot = po.tile([128, 512], HF, tag="ot")
        if evac == "s":
            nc.scalar.copy(ot[:], pso[:])
        else:
            nc.vector.tensor_copy(ot[:], pso[:])
        nc.sync.dma_start(
            out=out_d.ap()[r0:r0 + 128, f0:f0 + 512], in_=ot[:])

    def emit_scores(ic, jt):
        i0 = ic * 512
        j0 = jt * 128
        ps = ps2s.tile([128, 1024], F, tag="pss")
        nc.tensor.matmul(
            ps[:, 0:512], kT[0:64, j0:j0 + 128], qT[0:64, i0:i0 + 512],
            start=True, stop=True, tile_position=(0, 0),
        )
        nc.tensor.matmul(
            ps[:, 512:1024], kT[64:128, j0:j0 + 128],
            qT[64:128, i0:i0 + 512],
            start=True, stop=True, tile_position=(64, 0),
        )
        return ps

    def emit_exp(ps, eT, jt):
        if jt in DVE_JT:
            nc.vector.tensor_scalar(
                out=eT[:].bitcast(I16),
                in0=ps[:],
                scalar1=A_SCH,
                scalar2=B_SCH,
                op0=mybir.AluOpType.mult,
                op1=mybir.AluOpType.add,
            )
        else:
            nc.scalar.activation(eT[:], ps[:], Act.Exp, scale=0.125)

    # Two-step score skew. Tile encodes cross-engine deps as program-order
    # counters on each engine's progress semaphore, so an exp gated on its
    # scores transitively waits on EVERY PE instruction emitted earlier.
    # Emitting scores(n+2) (and the DVE exp prefetch) BEFORE vals(n) keeps
    # the exp stream's gate free of the exp(n)->vals(n) chain: ScalarE runs
    # back-to-back and DVE tiles become true holes. oproj units are emitted
    # at the top of an iteration so the PE fills its exp-wait stall with
    # them instead of idling.
    pending = None  # (ic, va0, va1, van) with normalize+out-proj still to emit
    from collections import deque
    ps_q = deque()
    eT_q = {}

    def emit_step(n):
        # scores + (for DVE tiles) the exp itself, two steps ahead
        jt_n = n % NJ
        ps_q.append(emit_scores(n // NJ, jt_n))
        if jt_n in DVE_JT:
            eT_q[n] = pe.tile([128, 1024], HF, tag="eT", name="eT_pf")
            emit_exp(ps_q[-1], eT_q[n], jt_n)

    emit_step(0)
    emit_step(1)
    for ic in range(NI):
        pv0 = ps2v.tile([DH + 1, 512], F, tag="pv0")
        pv1 = ps2v.tile([DH + 1, 512], F, tag="pv1")
        # out-proj units of the previous chunk, spread through this jt loop
        units = []
        if pending is not None:
            pic, _, _, pvan = pending
            units = [(pic, pvan, iw, fc)
                     for iw in range(4) for fc in range(F_OUT // 512)]
        for jt in range(NJ):
            n = ic * NJ + jt
            if jt % 2 == 1 and 5 <= jt < 21 and units:
                oproj_unit(*units.pop(0))
            ps = ps_q.popleft()
            if n in eT_q:
                eT = eT_q.pop(n)
            else:
                eT = pe.tile([128, 1024], HF, tag="eT")
                emit_exp(ps, eT, jt)
            if n + 2 < NI * NJ:
                emit_step(n + 2)
            nc.tensor.matmul(
                pv0[:], vx0[:, jt, :], eT[:, 0:512],
                start=(jt == 0), stop=(jt == NJ - 1),
            )
            nc.tensor.matmul(
                pv1[:], vx1[:, jt, :], eT[:, 512:1024],
                start=(jt == 0), stop=(jt == NJ - 1),
            )
            if pending is not None:
                if jt == 0:
                    rc_pend = norm_a(pending[0], pending[1], pending[2])
                elif jt == 1:
                    norm_b(pending[0], rc_pend, pending[1], pending[2],
                           pending[3])
        for u in units:
            oproj_unit(*u)

        # evacuate this chunk's vals+denominators: the next chunk's first
        # vals matmul reclaims the PSUM bank after one copy per head
        va0 = p2v.tile([DH + 1, 512], F, tag="va0")
        va1 = p2v.tile([DH + 1, 512], F, tag="va1")
        van = p2v.tile([D2, 512], HF, tag="van")
        nc.vector.tensor_copy(va0[:], pv0[:])
        nc.vector.tensor_copy(va1[:], pv1[:])
        pending = (ic, va0, va1, van)

    # final flush: alternate psum evacuation engines so the 2-buffer pso
    # chain pipelines instead of serializing on the DVE
    rc_fin = norm_a(pending[0], pending[1], pending[2])
    norm_b(pending[0], rc_fin, pending[1], pending[2], pending[3])
    k = 0
    for iw in range(4):
        for fc in range(F_OUT // 512):
            oproj_unit(pending[0], pending[3], iw, fc,
                       evac="s" if k % 2 else "v")
            k += 1


def _get_built():
    global _BUILT
    if _BUILT is None:
        _BUILT = _build()
    return _BUILT


def kernel(x, Wq, bq, Wk, bk, Wv, bv, Wo, bo):
    from concourse.bass_utils import run_bass_kernel_spmd

    x = np.asarray(x, dtype=np.float32)
    Wq = np.asarray(Wq, dtype=np.float32)
    Wk = np.asarray(Wk, dtype=np.float32)
    Wv = np.asarray(Wv, dtype=np.float32)
    Wo = np.asarray(Wo, dtype=np.float32)
    bq = np.asarray(bq, dtype=np.float32)
    bk = np.asarray(bk, dtype=np.float32)
    bv = np.asarray(bv, dtype=np.float32)
    bo = np.asarray(bo, dtype=np.float32)

    nc = _get_built()

    xT = np.ascontiguousarray(x.T.astype(np.float16))  # [F_IN, L]
    in_maps = []
    for c in range(NCORES):
        hs = slice(c * HPC, (c + 1) * HPC)
        in_maps.append({
            "xT": xT,
            "wq": np.ascontiguousarray(
                Wq[:, hs, :].reshape(F_IN, D2).astype(np.float16)),
            "wk": np.ascontiguousarray(
                Wk[:, hs, :].reshape(F_IN, D2).astype(np.float16)),
            "wv": np.ascontiguousarray(
                Wv[:, hs, :].reshape(F_IN, D2).astype(np.float16)),
            "bq": np.ascontiguousarray(bq[hs].reshape(D2)),
            "bk": np.ascontiguousarray(bk[hs].reshape(D2)),
            "wo": np.ascontiguousarray(
                Wo[hs].reshape(D2, F_OUT).astype(np.float16)),
        })

    res = run_bass_kernel_spmd(nc, in_maps, list(range(NCORES)))
    acc = np.zeros((L, F_OUT), dtype=np.float64)
    for c in range(NCORES):
        acc += res.results[c]["out"].astype(np.float64)
    # bv contribution (softmax rows sum to 1) + bo, both exact on host
    acc += (bv.reshape(1, H * DH).astype(np.float64)
            @ Wo.reshape(H * DH, F_OUT).astype(np.float64))
    acc += bo.astype(np.float64)
    return acc.astype(np.float32)


# revision 20
# speedup vs baseline: 1.0897x; 1.0132x over previous
"""Multi-head self-attention Trainium2 kernel (8 NeuronCores, head-parallel).

Problem: L=4096, F_IN=1024, H=16, DH=64, F_OUT=1024, fp32.
Sharding: 2 heads per core (tensor parallel over heads). Each core computes
its 2 heads' attention and its partial output projection; the host sums the
8 partials (the all-reduce of the sharding hint, done at gather time).

v2 changes vs the 394us baseline:
  * fp16 everywhere in SBUF (x, q, k, v, Wo, out): fp16 matmuls stream at
    1 cycle/row (the fp32r oproj/proj moving operands were 2 cyc/row), DMA
    bytes halve, and fp16's 2^-11 rounding buys error budget for the
    Schraudolph tiles below.
  * ~25% of the 256 exp tiles move off ScalarE (the old roofline at
    1.11us/tile) onto the idle VectorE as a one-instruction exp bit-trick:
    i16 = rint(A*s + B) written into the fp16 eT tile via an int16 bitcast
    view, where A = 0.125*log2(e)*1024, B = 15360-59. The int16 value IS
    the fp16 bit pattern of ~exp(s/8) (max rel err 4%, RMS 1.8%). The
    softmax denominator sums the same approximated weights (ones-column
    trick), so normalization stays consistent; only the weight *shape*
    within DVE-assigned j-tiles carries the sawtooth noise.
  * Output projection packs both heads into one K=128 matmul (normalized
    vals for head0/head1 land in rows 0:64/64:128 of one fp16 tile; Wo is
    host-packed to [128, F_OUT]), halving oproj matmuls and enabling FWL.
  * Phase 1 runs in 8 chunks of 512 with one batched x DMA per chunk
    (the old per-[128,512]-fp32 DMAs serialized ~40us on the sync queue).

v3 changes vs v2 (386us):
  * The DVE queue is 8-deep strict FIFO, and v2 clogged it: sem-waits and
    oproj evacuations sat ahead of the critical Schraudolph exps, which
    stalled vals -> scores-buffer reuse -> the ScalarE ACT stream (~123us
    of ACT gaps). v3 dedicates the DVE queue to exp + psum evacuations:
    - the Schraudolph exp for step n+1 is emitted one step early (right
      after its scores matmuls), so it runs concurrently with ACT(n);
    - the normalize chain's DMAs (denominator extract, reciprocal
      broadcast) move to the GpSimd DGE queue, off the Sync queue that
      out-DMAs block, and the gpsimd partition_broadcast becomes a
      stride-0-partition DMA so GpSimd<->DVE sem ping-pong disappears.
  * exp offload raised to 10/32 j-tiles (~31%).
  * Startup: x chunk-0 DMA issues before the wq/wo DMAs (first proj matmul
    ~12us earlier); tail: the final flush alternates its psum evacuations
    between ScalarE and VectorE so the 2-buffer pso chain pipelines.

Per-core pipeline:
  1. qT,kT [128,4096] fp16 = W.T @ x.T; v via PE transpose-mode into
     natural [j, d] layout vx (fp16, ones column appended for the softmax
     denominator).
  2. Per i-chunk (512), per j-tile (128): scoresT for both heads as
     row-packed K=64 matmuls -> exp over the [128,1024] psum pair on
     ScalarE (ACT, fp16 out) or VectorE (Schraudolph) -> attn@v with the
     ones-column so PSUM row 64 accumulates the denominators.
  3. Normalize + output projection interleave into the next chunk's j-loop:
     evacuate vals fp32, reciprocal_approx_fast on the DMA-extracted
     denominators, gpsimd broadcast, normalize into packed fp16 rows,
     one K=128 matmul per (i-tile, f-chunk), fp16 DMA out.

Bias handling: bq/bk folded into the ACT bias at qT/kT evacuation; bv exact
on host (softmax rows sum to 1 => out += sum_h bv_h @ Wo_h); bo on host.
"""

import numpy as np

L, F_IN, H, DH, F_OUT = 4096, 1024, 16, 64, 1024
NCORES = 8
HPC = H // NCORES  # heads per core = 2
D2 = HPC * DH      # 128, per-core packed head dim

_BUILT = None

# fp16-bit-space Schraudolph exp: bits = rint(A*s + B) ~ fp16(exp(0.125*s))
A_SCH = 0.125 * float(np.log2(np.e)) * 1024.0
B_SCH = 15360.0 - 59.0
# j-tiles whose exp runs on VectorE (rest on ScalarE). Adjacent pairs:
# each DVE tile leaves a ~1us hole in the ScalarE stream (its gate chain
# runs through the PE FIFO), but a pair shares one hole.
DVE_JT = frozenset((2, 3, 10, 11, 18, 19, 26, 27))


def _build():
    import concourse.bass as bass  # noqa: F401
    import concourse.mybir as mybir
    import concourse.tile as tile
    from concourse import bacc
    from concourse.masks import make_identity

    F = mybir.dt.float32
    HF = mybir.dt.float16
    I16 = mybir.dt.int16
    Act = mybir.ActivationFunctionType

    nc = bacc.Bacc("TRN2", target_bir_lowering=False, debug=False)

    xT_d = nc.declare_dram_parameter("xT", [F_IN, L], HF, isOutput=False)
    wq_d = nc.declare_dram_parameter("wq", [F_IN, D2], HF, isOutput=False)
    wk_d = nc.declare_dram_parameter("wk", [F_IN, D2], HF, isOutput=False)
    wv_d = nc.declare_dram_parameter("wv", [F_IN, D2], HF, isOutput=False)
    bq_d = nc.declare_dram_parameter("bq", [D2], F, isOutput=False)
    bk_d = nc.declare_dram_parameter("bk", [D2], F, isOutput=False)
    wo_d = nc.declare_dram_parameter("wo", [D2, F_OUT], HF, isOutput=False)
    out_d = nc.declare_dram_parameter("out", [L, F_OUT], HF, isOutput=True)

    KT = F_IN // 128   # 8 f-tiles
    NI = L // 512      # 8 i-chunks
    NJ = L // 128      # 32 j-tiles
    CL = 512           # phase-1 chunk width in L
    NC = L // CL       # 8 chunks

    with tile.TileContext(nc) as tc:
        with tc.tile_pool(name="persist", bufs=1) as pp:
            qT = pp.tile([128, L], HF, tag="qT")             # [d2, i]
            kT = pp.tile([128, L], HF, tag="kT")             # [d2, j]
            vx0 = pp.tile([128, NJ, DH + 1], HF, tag="vx0")  # [j_in, jt, d|1]
            vx1 = pp.tile([128, NJ, DH + 1], HF, tag="vx1")
            bq = pp.tile([128, 1], F, tag="bq")
            bk = pp.tile([128, 1], F, tag="bk")
            ones32 = pp.tile([128, NJ], HF, tag="ones32")
            warm = pp.tile([1, 1], F, tag="warm")
            warm2 = pp.tile([1, 1], F, tag="warm2")

            # pre-warm the exp table set + the gpsimd DGE while DMAs run
            nc.vector.memset(warm[:], 0.0)
            nc.scalar.activation(warm[:], warm[:], Act.Exp, scale=1.0)
            nc.gpsimd.dma_start(out=warm2[:], in_=warm[:])

            nc.vector.memset(ones32[:], 1.0)
            nc.vector.tensor_copy(vx0[:, :, DH:DH + 1], ones32[:, :, None])
            nc.vector.tensor_copy(vx1[:, :, DH:DH + 1], ones32[:, :, None])

            # Pools for the attention phase are opened before phase 1 is
            # emitted so the scheduler can overlap the phase-1 tail with
            # early score matmuls (PSUM: ps2s 4 + ps2v 2 + phase1 2 = 8).
            with tc.tile_pool(name="p2", bufs=1) as p2, \
                 tc.tile_pool(name="p2v", bufs=2) as p2v, \
                 tc.tile_pool(name="expp", bufs=6) as pe, \
                 tc.tile_pool(name="outp", bufs=4) as po, \
                 tc.tile_pool(name="ps2s", bufs=2, space="PSUM") as ps2s, \
                 tc.tile_pool(name="ps2v", bufs=1, space="PSUM") as ps2v:
                wo = p2.tile([D2, F_OUT], HF, tag="wo")

                # ---- Phase 1: QKV projections over 8 chunks of L ----
                with tc.tile_pool(name="p1w", bufs=1) as p1w, \
                     tc.tile_pool(name="p1x", bufs=2) as p1x, \
                     tc.tile_pool(name="ps1", bufs=2, space="PSUM") as ps1:
                    wq = p1w.tile([128, KT, D2], HF, tag="wq")
                    wk = p1w.tile([128, KT, D2], HF, tag="wk")
                    wv = p1w.tile([128, KT, D2], HF, tag="wv")
                    ident = p1w.tile([128, 128], HF, tag="ident")
                    # DMA order matters for the pipeline ramp: the first
                    # k-projection needs wk + x chunk 0, so everything else
                    # queues behind those two.
                    nc.sync.dma_start(
                        out=wk[:],
                        in_=wk_d.ap().rearrange("(k p) d -> p k d", p=128),
                    )
                    make_identity(nc, ident[:])
                    nc.sync.dma_start(out=bk[:], in_=bk_d.ap()[:, None])
                    nc.sync.dma_start(out=bq[:], in_=bq_d.ap()[:, None])

                    def proj(wt, dst, bias, xt, g0):
                        ps = ps1.tile([128, 512], F, tag="ps1")
                        for kt in range(KT):
                            nc.tensor.matmul(
                                ps[:], wt[:, kt, :], xt[:, kt, :],
                                start=(kt == 0), stop=(kt == KT - 1),
                            )
                        if bias is not None:
                            nc.scalar.activation(
                                dst[:, g0:g0 + 512], ps[:], Act.Identity,
                                bias=bias[:], scale=1.0,
                            )
                        else:
                            nc.scalar.copy(dst[:, 0:512], ps[:])

                    for ch in range(NC):
                        l0 = ch * CL
                        xt = p1x.tile([128, KT, CL], HF, tag="xt")
                        nc.sync.dma_start(
                            out=xt[:],
                            in_=xT_d.ap().rearrange(
                                "(k p) l -> p k l", p=128)[:, :, l0:l0 + CL],
                        )
                        if ch == 0:
                            for wt, wd in ((wv, wv_d), (wq, wq_d)):
                                nc.sync.dma_start(
                                    out=wt[:],
                                    in_=wd.ap().rearrange(
                                        "(k p) d -> p k d", p=128),
                                )
                            nc.sync.dma_start(out=wo[:], in_=wo_d.ap())
                        vTq = p1x.tile([128, CL], HF, tag="vTq")
                        # k and v unlock this chunk's score matmuls; qT is
                        # ramp-critical only in chunk 0 (i-chunk 0 columns),
                        # so later chunks emit the q projection last.
                        proj(wk, kT, bk, xt, l0)
                        proj(wv, vTq, None, xt, l0)
                        if ch == 0:
                            proj(wq, qT, bq, xt, l0)
                        for jl in range(CL // 128):
                            jt = ch * (CL // 128) + jl
                            pt = ps1.tile([128, 1024], HF, tag="ps1")
                            nc.tensor.transpose(
                                pt[:, 0:128],
                                vTq[:, jl * 128:(jl + 1) * 128], ident[:])
                            nc.vector.tensor_copy(vx0[:, jt, 0:DH], pt[:, 0:DH])
                            nc.vector.tensor_copy(vx1[:, jt, 0:DH],
                                                  pt[:, DH:D2])
                        if ch != 0:
                            proj(wq, qT, bq, xt, l0)

                # ---- Phase 2+3: attention, interleaved normalize/out-proj ----
                with tc.tile_pool(name="ps2o", bufs=2, space="PSUM") as ps2o:
                    _phase2(nc, ps2s, ps2v, ps2o, pe, po, p2v,
                            qT, kT, vx0, vx1, wo, out_d,
                            NI, NJ, F, HF, I16, Act, mybir)

    nc.compile()
    return nc


def _phase2(nc, ps2s, ps2v, ps2o, pe, po, p2v, qT, kT, vx0, vx1, wo,
            out_d, NI, NJ, F, HF, I16, Act, mybir):
    def norm_a(ic, va0, va1):
        # denominator extract + reciprocal for chunk ic. The extraction
        # DMAs ride the gpsimd DGE queue so out-DMA sem waits on the sync
        # queue cannot delay them.
        sh = p2v.tile([1, 1024], F, tag="sh")
        rc = p2v.tile([1, 1024], F, tag="rc")
        nc.gpsimd.dma_start(out=sh[:, 0:512], in_=va0[DH:DH + 1, :])
        nc.gpsimd.dma_start(out=sh[:, 512:1024], in_=va1[DH:DH + 1, :])
        nc.vector.reciprocal_approx_fast(out=rc[:], in_=sh[:])
        return rc

    def norm_b(ic, rc, va0, va1, van):
        # broadcast + normalize into packed fp16 rows (h0 0:64, h1 64:128),
        # emitted one step after norm_a so the gpsimd/DVE handoffs never
        # head-block the DVE queue ahead of a prefetched exp.
        for va, r0, c0, tg in ((va0, 0, 0, "0"), (va1, DH, 512, "1")):
            rb = p2v.tile([DH, 512], F, tag="rb" + tg)
            nc.gpsimd.partition_broadcast(rb[:], rc[0:1, c0:c0 + 512],
                                          channels=DH)
            nc.vector.tensor_mul(van[r0:r0 + DH, :], va[0:DH, :], rb[:])

    def oproj_unit(ic, van, iw, fc, evac="v"):
        # one output-projection tile of chunk ic: both heads in one K=128 mm
        isl = slice(iw * 128, (iw + 1) * 128)
        r0 = ic * 512 + iw * 128
        f0 = fc * 512
        pso = ps2o.tile([128, 512], F, tag="pso")
        nc.tensor.matmul(
            pso[:], van[:, isl], wo[:, f0:f0 + 512],
            start=True, stop=True,
        )
        ot = po.tile([128, 512], HF, tag="ot")
        if evac == "s":
            nc.scalar.copy(ot[:], pso[:])
        else:
            nc.vector.tensor_copy(ot[:], pso[:])
        nc.sync.dma_start(
            out=out_d.ap()[r0:r0 + 128, f0:f0 + 512], in_=ot[:])

    def emit_scores(ic, jt):
        i0 = ic * 512
        j0 = jt * 128
        ps = ps2s.tile([128, 1024], F, tag="pss")
        nc.tensor.matmul(
            ps[:, 0:512], kT[0:64, j0:j0 + 128], qT[0:64, i0:i0 + 512],
            start=True, stop=True, tile_position=(0, 0),
        )
        nc.tensor.matmul(
            ps[:, 512:1024], kT[64:128, j0:j0 + 128],
            qT[64:128, i0:i0 + 512],
            start=True, stop=True, tile_position=(64, 0),
        )
        return ps

    def emit_exp(ps, eT, jt):
        if jt in DVE_JT:
            nc.vector.tensor_scalar(
                out=eT[:].bitcast(I16),
                in0=ps[:],
                scalar1=A_SCH,
                scalar2=B_SCH,
                op0=mybir.AluOpType.mult,
                op1=mybir.AluOpType.add,
            )
        else:
            nc.scalar.activation(eT[:], ps[:], Act.Exp, scale=0.125)

    # Two-step score skew. Tile encodes cross-engine deps as program-order
    # counters on each engine's progress semaphore, so an exp gated on its
    # scores transitively waits on EVERY PE instruction emitted earlier.
    # Emitting scores(n+2) (and the DVE exp prefetch) BEFORE vals(n) keeps
    # the exp stream's gate free of the exp(n)->vals(n) chain: ScalarE runs
    # back-to-back and DVE tiles become true holes. oproj units are emitted
    # at the top of an iteration so the PE fills its exp-wait stall with
    # them instead of idling.
    pending = None  # (ic, va0, va1, van) with normalize+out-proj still to emit
    from collections import deque
    ps_q = deque()
    eT_q = {}

    def emit_step(n):
        # scores + (for DVE tiles) the exp itself, two steps ahead
        jt_n = n % NJ
        ps_q.append(emit_scores(n // NJ, jt_n))
        if jt_n in DVE_JT:
            eT_q[n] = pe.tile([128, 1024], HF, tag="eT", name="eT_pf")
            emit_exp(ps_q[-1], eT_q[n], jt_n)

    emit_step(0)
    emit_step(1)
    for ic in range(NI):
        pv0 = ps2v.tile([DH + 1, 512], F, tag="pv0")
        pv1 = ps2v.tile([DH + 1, 512], F, tag="pv1")
        # out-proj units of the previous chunk, spread through this jt loop
        units = []
        if pending is not None:
            pic, _, _, pvan = pending
            units = [(pic, pvan, iw, fc)
                     for iw in range(4) for fc in range(F_OUT // 512)]
        for jt in range(NJ):
            n = ic * NJ + jt
            if jt % 2 == 1 and 9 <= jt < 25 and units:
                oproj_unit(*units.pop(0))
            ps = ps_q.popleft()
            if n in eT_q:
                eT = eT_q.pop(n)
            else:
                eT = pe.tile([128, 1024], HF, tag="eT")
                emit_exp(ps, eT, jt)
            if n + 2 < NI * NJ:
                emit_step(n + 2)
            nc.tensor.matmul(
                pv0[:], vx0[:, jt, :], eT[:, 0:512],
                start=(jt == 0), stop=(jt == NJ - 1),
            )
            nc.tensor.matmul(
                pv1[:], vx1[:, jt, :], eT[:, 512:1024],
                start=(jt == 0), stop=(jt == NJ - 1),
            )
            if pending is not None:
                if jt == 0:
                    rc_pend = norm_a(pending[0], pending[1], pending[2])
                elif jt == 1:
                    norm_b(pending[0], rc_pend, pending[1], pending[2],
                           pending[3])
        for u in units:
            oproj_unit(*u)

        # evacuate this chunk's vals+denominators on BOTH evac-capable
        # engines concurrently: the next chunk's first vals matmuls reclaim
        # the pv banks ~0.7us after vals(31) instead of ~1.4us, and the PE
        # never idles long enough for HAM to re-throttle
        va0 = p2v.tile([DH + 1, 512], F, tag="va0")
        va1 = p2v.tile([DH + 1, 512], F, tag="va1")
        van = p2v.tile([D2, 512], HF, tag="van")
        nc.scalar.copy(va0[:], pv0[:])
        nc.vector.tensor_copy(va1[:], pv1[:])
        pending = (ic, va0, va1, van)

    # final flush: alternate psum evacuation engines so the 2-buffer pso
    # chain pipelines instead of serializing on the DVE
    rc_fin = norm_a(pending[0], pending[1], pending[2])
    norm_b(pending[0], rc_fin, pending[1], pending[2], pending[3])
    k = 0
    for iw in range(4):
        for fc in range(F_OUT // 512):
            oproj_unit(pending[0], pending[3], iw, fc,
                       evac="s" if k % 2 else "v")
            k += 1


def _get_built():
    global _BUILT
    if _BUILT is None:
        _BUILT = _build()
    return _BUILT


def kernel(x, Wq, bq, Wk, bk, Wv, bv, Wo, bo):
    from concourse.bass_utils import run_bass_kernel_spmd

    x = np.asarray(x, dtype=np.float32)
    Wq = np.asarray(Wq, dtype=np.float32)
    Wk = np.asarray(Wk, dtype=np.float32)
    Wv = np.asarray(Wv, dtype=np.float32)
    Wo = np.asarray(Wo, dtype=np.float32)
    bq = np.asarray(bq, dtype=np.float32)
    bk = np.asarray(bk, dtype=np.float32)
    bv = np.asarray(bv, dtype=np.float32)
    bo = np.asarray(bo, dtype=np.float32)

    nc = _get_built()

    xT = np.ascontiguousarray(x.T.astype(np.float16))  # [F_IN, L]
    in_maps = []
    for c in range(NCORES):
        hs = slice(c * HPC, (c + 1) * HPC)
        in_maps.append({
            "xT": xT,
            "wq": np.ascontiguousarray(
                Wq[:, hs, :].reshape(F_IN, D2).astype(np.float16)),
            "wk": np.ascontiguousarray(
                Wk[:, hs, :].reshape(F_IN, D2).astype(np.float16)),
            "wv": np.ascontiguousarray(
                Wv[:, hs, :].reshape(F_IN, D2).astype(np.float16)),
            "bq": np.ascontiguousarray(bq[hs].reshape(D2)),
            "bk": np.ascontiguousarray(bk[hs].reshape(D2)),
            "wo": np.ascontiguousarray(
                Wo[hs].reshape(D2, F_OUT).astype(np.float16)),
        })

    res = run_bass_kernel_spmd(nc, in_maps, list(range(NCORES)))
    acc = np.zeros((L, F_OUT), dtype=np.float64)
    for c in range(NCORES):
        acc += res.results[c]["out"].astype(np.float64)
    # bv contribution (softmax rows sum to 1) + bo, both exact on host
    acc += (bv.reshape(1, H * DH).astype(np.float64)
            @ Wo.reshape(H * DH, F_OUT).astype(np.float64))
    acc += bo.astype(np.float64)
    return acc.astype(np.float32)
